# revision 14
# baseline (speedup 1.0000x reference)
"""Trainium2 Bass kernel for per-sample argmax-histogram (nn_BasicCount).

Input : full  x [64, 16384, 100] f32
Output: full  freqs [64, 100] f32  (per-sample normalized histogram of
        argmax over classes)

Sharding: pure data parallel — batch dim split 8 ways across the 8
NeuronCores (8 samples per core), no communication.

v2 algorithm (per core, shapes hardcoded):
  Work units of up to 4096 positions laid out [128 partitions x k groups
  x 100 classes].  Two tile flavors:

  Flavor B (int16 2x path, most tiles):
    1. ACT: q = int16(x * 4096) as ONE full-tile Copy instruction
       (monotone quantization; quantum 2.44e-4 -> false-tie rate
       ~3e-4/position, ~0.05 expected stray counts per bin — far
       under the 2e-2 rel-err gate).
    2. DVE: m3i[p,k] = max_c q — segmented tensor_reduce on int16.
    3. DVE (tiny): mrep4[p, k*4+j] = m3i[p,k] (packed 4x replica).
    4. DVE: mask = is_lt(q, mrep4) viewed [P, k, 25, 4] so every
       operand's innermost AP dim is packed 2-byte stride-1 -> DVE
       2x mode (the m-broadcast rides a middle 0-stride dim).
  Flavor A (legacy ACT path, for engine balance):
    1. DVE: m3 = tensor_reduce(max) on fp32 x.
    2. ACT: per-group Sign(m - x) instrs (FD=100, ~271 ns each).
  Both produce a bf16 complement mask [x < m]; PE accumulates
  per-sample mask sums into one PSUM bank [8, 400]; finale folds the
  4 k-subgroups, freqs = 1 - S/N.

Engine model (HW-measured baseline): DVE 1x = (FD+~130)/0.96 ns; ACT
per-instr = (FD+~160)/0.96 ns; DMA union busy 152.8 us (the roofline).
Flavor B: ACT 3.4us + DVE ~5.5us per tile; flavor A: ACT 8.67, DVE
3.45.  Mix ~7 A-tiles to balance both engines near the DMA floor.

Dead ends (HW-measured, do not revisit): GpSimd offload (SBUF
contention halves DVE), fp32 PE subtract (PE critical), bf16/fp16
value compares (false argmax ties above the 2e-2 gate), custom DVE ops
(1x only).  int16 compares are exact on the quantized grid, so the
2e-2 risk analysis is Poisson tie-counting, not fp rounding.
"""

import sys

if "/opt/trn_rl_repo" not in sys.path:
    sys.path.insert(0, "/opt/trn_rl_repo")

from contextlib import ExitStack

import numpy as np

import concourse.bacc as bacc
import concourse.bass as bass
import concourse.tile as tile
from concourse import mybir
from concourse.bass_utils import run_bass_kernel_spmd

B, N, C = 64, 16384, 100
NCORES = 8
SPB = B // NCORES  # samples per core = 8
P = 128  # partitions
POS_PER_TILE = 4096
K = POS_PER_TILE // P  # position groups per partition = 32
F = K * C  # free size per tile = 3200
TILES_PER_SAMPLE = N // POS_PER_TILE  # 4
NTILES = SPB * TILES_PER_SAMPLE  # 32
QCHUNK = 400  # matmul rhs free chunk (4 groups x 100 classes)
QSCALE = 4096.0  # int16 quantization scale; |x| < 8 guaranteed


def _schedule(variant):
    """Work units: (sample, n0, npos, flavor); flavor in {a, b, d}.

    a = ACT per-group Sign masks (fp32 exact)
    b = int16 2x DVE masks (ACT constructs q)
    d = fp32 DVE TT is_lt masks (legacy dve flavor)
    """
    units = []
    SUB = 1024
    if variant == "legacy":
        mid_pat = [
            "a", "d", "a", "d", "a", "a", "d", "a", "d", "a",
            "a", "d", "a", "d", "a", "a", "d", "a", "d", "a",
            "a", "d", "a", "d", "a", "a", "d", "d", "a", "d",
        ]
        ramp = ["a", "d", "a", "d"]
        drain = ["d", "d", "d", "d"]
    elif variant == "allb":
        mid_pat = ["b"] * 30
        ramp = ["b"] * 4
        drain = ["b"] * 4
    else:  # v2: 7 A-tiles spread among B-tiles (a ~ 7 balances engines),
        # none near the end so the post-DMA drain is all cheap B-tiles
        mid_pat = list("bbabbabbbabbbabbabbbabbabbbbbb")
        ramp = ["b", "a", "b", "b"]
        drain = ["b"] * 4
    assert len(mid_pat) == 30
    for j, e in enumerate(ramp):
        units.append((0, j * SUB, SUB, e))
    for i in range(1, NTILES - 1):
        s = i // TILES_PER_SAMPLE
        n0 = (i % TILES_PER_SAMPLE) * POS_PER_TILE
        units.append((s, n0, POS_PER_TILE, mid_pat[i - 1]))
    base = (TILES_PER_SAMPLE - 1) * POS_PER_TILE
    for j, e in enumerate(drain):
        units.append((SPB - 1, base + j * SUB, SUB, e))
    return units


def build_bass(variant: str = "v2", bufs: int = 9):
    fp32 = mybir.dt.float32
    bf16 = mybir.dt.bfloat16
    i16 = mybir.dt.int16

    units = _schedule(variant)

    nc = bacc.Bacc(None)
    x_in = nc.declare_dram_parameter("input", [SPB, N, C], fp32, isOutput=False)
    out_d = nc.declare_dram_parameter("freqs", [SPB, C], fp32, isOutput=True)

    with ExitStack() as ctx:
        tc = ctx.enter_context(tile.TileContext(nc))
        xp = ctx.enter_context(tc.tile_pool(name="x", bufs=bufs))
        qp = ctx.enter_context(tc.tile_pool(name="q", bufs=7))
        trp = ctx.enter_context(tc.tile_pool(name="tr", bufs=4))
        mp_max = ctx.enter_context(tc.tile_pool(name="m", bufs=8))
        mrp = ctx.enter_context(tc.tile_pool(name="mrep", bufs=7))
        mp = ctx.enter_context(tc.tile_pool(name="mask", bufs=4))
        singles = ctx.enter_context(tc.tile_pool(name="singles", bufs=1))
        psum = ctx.enter_context(tc.tile_pool(name="psum", bufs=1, space="PSUM"))

        # per-sample matmul selectors: sel[:, s, :] is [128, 8] with col s = 1
        sel = singles.tile([P, SPB, SPB], bf16)
        nc.vector.memset(sel, 0.0)
        for s in range(SPB):
            nc.vector.memset(sel[:, s, s : s + 1], 1.0)

        # Warm the ScalarE Sign activation table (~2.7 us load+drain)
        # before the first real mask depends on it.
        warm = singles.tile([P, 2], fp32)
        nc.vector.memset(warm[:, 0:1], 0.0)
        nc.scalar.activation(
            out=warm[:, 1:2],
            in_=warm[:, 0:1],
            func=mybir.ActivationFunctionType.Sign,
        )

        acc = psum.tile([SPB, QCHUNK], fp32)  # one PSUM bank, [8, 400]

        total_mm = sum(np_ * C // (P * QCHUNK) for _, _, np_, _ in units)
        nu = len(units)
        xts = [None] * nu
        qts = [None] * nu
        m3s = [None] * nu
        mr4s = [None] * nu

        def issue_load(i):
            s, n0, npos, _ = units[i]
            k = npos // P
            f = k * C
            xt = xp.tile([P, f], fp32, tag="x")
            src = x_in[s, n0 : n0 + npos, :].rearrange("(p k) c -> p (k c)", p=P)
            nc.sync.dma_start(out=xt, in_=src)
            xts[i] = xt

        def issue_prep(i):
            """Construct (B) + reduce; engine streams stay mostly parallel."""
            s, n0, npos, fl = units[i]
            k = npos // P
            f = k * C
            xt = xts[i]
            if fl == "b":
                qt = qp.tile([P, f], i16, tag="q")
                nc.scalar.activation(
                    out=qt,
                    in_=xt,
                    func=mybir.ActivationFunctionType.Copy,
                    scale=QSCALE,
                )
                qts[i] = qt
                # 2x max tree: contiguous half-splits keep operands packed
                q3 = qt.rearrange("p (k c) -> p k c", c=C)
                r1 = trp.tile([P, k, 50], i16, tag="r1")
                nc.vector.tensor_tensor(
                    out=r1,
                    in0=q3[:, :, 0:50],
                    in1=q3[:, :, 50:100],
                    op=mybir.AluOpType.max,
                )
                r2 = trp.tile([P, k, 25], i16, tag="r2")
                nc.vector.tensor_tensor(
                    out=r2,
                    in0=r1[:, :, 0:25],
                    in1=r1[:, :, 25:50],
                    op=mybir.AluOpType.max,
                )
                m3 = mp_max.tile([P, k, 1], i16, tag="m")
                nc.vector.tensor_reduce(
                    out=m3,
                    in_=r2,
                    axis=mybir.AxisListType.X,
                    op=mybir.AluOpType.max,
                )
                m3s[i] = m3
                # packed 4x replica of the per-group max; tiny op on the
                # otherwise-idle GpSimd engine (256B/tile SBUF traffic)
                mr4 = mrp.tile([P, k, 4], i16, tag="mr")
                nc.gpsimd.tensor_scalar(
                    out=mr4,
                    in0=m3.broadcast_to([P, k, 4]),
                    scalar1=0,
                    scalar2=None,
                    op0=mybir.AluOpType.add,
                )
                mr4s[i] = mr4
            else:
                m3 = mp_max.tile([P, k, 1], fp32, tag="m")
                nc.vector.tensor_reduce(
                    out=m3,
                    in_=xt.rearrange("p (k c) -> p k c", c=C),
                    axis=mybir.AxisListType.X,
                    op=mybir.AluOpType.max,
                )
                m3s[i] = m3

        LA_DMA = 8
        LA_PREP = 5
        for i in range(min(LA_DMA, nu)):
            issue_load(i)
        for i in range(min(LA_PREP, nu)):
            issue_prep(i)

        mm = 0
        for i in range(nu):
            if i + LA_DMA < nu:
                issue_load(i + LA_DMA)
            if i + LA_PREP < nu:
                issue_prep(i + LA_PREP)
            s, n0, npos, fl = units[i]
            k = npos // P
            f = k * C
            nq = f // QCHUNK
            m3 = m3s[i]

            mask = mp.tile([P, f], bf16, tag="mask")
            if fl == "b":
                qt = qts[i]
                q4 = qt.rearrange("p (k c4 j) -> p k c4 j", k=k, c4=C // 4, j=4)
                mask4 = mask.rearrange(
                    "p (k c4 j) -> p k c4 j", k=k, c4=C // 4, j=4
                )
                mr4 = mr4s[i]
                m_b = mr4.rearrange("p (k one) j -> p k one j", one=1).broadcast_to(
                    [P, k, C // 4, 4]
                )
                nc.vector.tensor_tensor(
                    out=mask4,
                    in0=q4,
                    in1=m_b,
                    op=mybir.AluOpType.is_lt,
                )
            elif fl == "a":
                xt = xts[i]
                x3 = xt.rearrange("p (k c) -> p k c", c=C)
                mask3 = mask.rearrange("p (k c) -> p k c", c=C)
                for j in range(k):
                    nc.scalar.activation(
                        out=mask3[:, j, :],
                        in_=x3[:, j, :],
                        func=mybir.ActivationFunctionType.Sign,
                        bias=m3[:, j, :],
                        scale=-1.0,
                    )
            else:  # d: fp32 DVE TT
                xt = xts[i]
                x3 = xt.rearrange("p (k c) -> p k c", c=C)
                mask3 = mask.rearrange("p (k c) -> p k c", c=C)
                m_b = m3.broadcast_to([P, k, C])
                nc.vector.tensor_tensor(
                    out=mask3,
                    in0=x3,
                    in1=m_b,
                    op=mybir.AluOpType.is_lt,
                )

            for q in range(nq):
                nc.tensor.matmul(
                    acc,
                    sel[:, s, :],
                    mask[:, q * QCHUNK : (q + 1) * QCHUNK],
                    start=(mm == 0),
                    stop=(mm == total_mm - 1),
                )
                mm += 1

        # ---- finale: fold the 4 k-subgroups, freqs = 1 - S/N ----
        S = singles.tile([SPB, C], fp32)
        nc.vector.tensor_reduce(
            out=S,
            in_=acc.rearrange("p (g c) -> p c g", c=C),
            axis=mybir.AxisListType.X,
            op=mybir.AluOpType.add,
        )

        fq = singles.tile([SPB, C], fp32)
        nc.vector.tensor_scalar(
            out=fq,
            in0=S,
            scalar1=-1.0 / N,
            scalar2=1.0,
            op0=mybir.AluOpType.mult,
            op1=mybir.AluOpType.add,
        )

        nc.sync.dma_start(out=out_d[:, :], in_=fq)

    nc.finalize()
    return nc


_NC_CACHE = None


def _get_nc():
    global _NC_CACHE
    if _NC_CACHE is None:
        _NC_CACHE = build_bass()
    return _NC_CACHE


def run(inputs: dict, trace: bool = False, nc=None):
    """Shard, run on 8 cores, gather. Returns (freqs [64,100] f32, results)."""
    x = np.ascontiguousarray(np.asarray(inputs["input"], dtype=np.float32))
    assert x.shape == (B, N, C), x.shape
    if nc is None:
        nc = _get_nc()
    in_maps = [
        {"input": x[core * SPB : (core + 1) * SPB]} for core in range(NCORES)
    ]
    res = run_bass_kernel_spmd(nc, in_maps, list(range(NCORES)), trace=trace)
    out = np.concatenate([res.results[core]["freqs"] for core in range(NCORES)], axis=0)
    return out.astype(np.float32), res


def kernel(**inputs) -> np.ndarray:
    out, _ = run(inputs)
    return out


# revision 15
# speedup vs baseline: 1.1381x; 1.1381x over previous
"""Trainium2 Bass kernel for per-sample argmax-histogram (nn_BasicCount).

Input : full  x [64, 16384, 100] f32
Output: full  freqs [64, 100] f32  (per-sample normalized histogram of
        argmax over classes)

Sharding: pure data parallel — batch dim split 8 ways across the 8
NeuronCores (8 samples per core), no communication.

v2 algorithm (per core, shapes hardcoded):
  Work units of up to 4096 positions laid out [128 partitions x k groups
  x 100 classes].  Two tile flavors:

  Flavor B (int16 2x path, most tiles):
    1. ACT: q = int16(x * 4096) as ONE full-tile Copy instruction
       (monotone quantization; quantum 2.44e-4 -> false-tie rate
       ~3e-4/position, ~0.05 expected stray counts per bin — far
       under the 2e-2 rel-err gate).
    2. DVE: m3i[p,k] = max_c q — segmented tensor_reduce on int16.
    3. DVE (tiny): mrep4[p, k*4+j] = m3i[p,k] (packed 4x replica).
    4. DVE: mask = is_lt(q, mrep4) viewed [P, k, 25, 4] so every
       operand's innermost AP dim is packed 2-byte stride-1 -> DVE
       2x mode (the m-broadcast rides a middle 0-stride dim).
  Flavor A (legacy ACT path, for engine balance):
    1. DVE: m3 = tensor_reduce(max) on fp32 x.
    2. ACT: per-group Sign(m - x) instrs (FD=100, ~271 ns each).
  Both produce a bf16 complement mask [x < m]; PE accumulates
  per-sample mask sums into one PSUM bank [8, 400]; finale folds the
  4 k-subgroups, freqs = 1 - S/N.

Engine model (HW-measured baseline): DVE 1x = (FD+~130)/0.96 ns; ACT
per-instr = (FD+~160)/0.96 ns; DMA union busy 152.8 us (the roofline).
Flavor B: ACT 3.4us + DVE ~5.5us per tile; flavor A: ACT 8.67, DVE
3.45.  Mix ~7 A-tiles to balance both engines near the DMA floor.

Dead ends (HW-measured, do not revisit): GpSimd offload (SBUF
contention halves DVE), fp32 PE subtract (PE critical), bf16/fp16
value compares (false argmax ties above the 2e-2 gate), custom DVE ops
(1x only).  int16 compares are exact on the quantized grid, so the
2e-2 risk analysis is Poisson tie-counting, not fp rounding.
"""

import sys

if "/opt/trn_rl_repo" not in sys.path:
    sys.path.insert(0, "/opt/trn_rl_repo")

from contextlib import ExitStack

import numpy as np

import concourse.bacc as bacc
import concourse.bass as bass
import concourse.tile as tile
from concourse import mybir
from concourse.bass_utils import run_bass_kernel_spmd

B, N, C = 64, 16384, 100
NCORES = 8
SPB = B // NCORES  # samples per core = 8
P = 128  # partitions
POS_PER_TILE = 4096
K = POS_PER_TILE // P  # position groups per partition = 32
F = K * C  # free size per tile = 3200
TILES_PER_SAMPLE = N // POS_PER_TILE  # 4
NTILES = SPB * TILES_PER_SAMPLE  # 32
QCHUNK = 400  # matmul rhs free chunk (4 groups x 100 classes)
QSCALE = 4096.0  # int16 quantization scale; |x| < 8 guaranteed


def _schedule(variant):
    """Work units: (sample, n0, npos, flavor); flavor in {a, b, d}.

    a = ACT per-group Sign masks (fp32 exact)
    b = int16 2x DVE masks (ACT constructs q)
    d = fp32 DVE TT is_lt masks (legacy dve flavor)
    """
    units = []
    SUB = 1024
    if variant == "legacy":
        mid_pat = [
            "a", "d", "a", "d", "a", "a", "d", "a", "d", "a",
            "a", "d", "a", "d", "a", "a", "d", "a", "d", "a",
            "a", "d", "a", "d", "a", "a", "d", "d", "a", "d",
        ]
        ramp = ["a", "d", "a", "d"]
        drain = ["d", "d", "d", "d"]
    elif variant == "allb":
        mid_pat = ["b"] * 30
        ramp = ["b"] * 4
        drain = ["b"] * 4
    else:  # v2: 7 A-tiles spread among B-tiles (a ~ 7 balances engines),
        # none near the end so the post-DMA drain is all cheap B-tiles
        mid_pat = list("bbabbabbbabbbabbabbbabbabbbbbb")
        ramp = ["b", "a", "b", "b"]
        drain = ["b"] * 4
    assert len(mid_pat) == 30
    for j, e in enumerate(ramp):
        units.append((0, j * SUB, SUB, e))
    for i in range(1, NTILES - 1):
        s = i // TILES_PER_SAMPLE
        n0 = (i % TILES_PER_SAMPLE) * POS_PER_TILE
        units.append((s, n0, POS_PER_TILE, mid_pat[i - 1]))
    base = (TILES_PER_SAMPLE - 1) * POS_PER_TILE
    for j, e in enumerate(drain):
        units.append((SPB - 1, base + j * SUB, SUB, e))
    return units


def build_bass(variant: str = "v2", bufs: int = 9):
    fp32 = mybir.dt.float32
    bf16 = mybir.dt.bfloat16
    i16 = mybir.dt.int16

    units = _schedule(variant)

    nc = bacc.Bacc(None)
    x_in = nc.declare_dram_parameter("input", [SPB, N, C], fp32, isOutput=False)
    out_d = nc.declare_dram_parameter("freqs", [SPB, C], fp32, isOutput=True)

    with ExitStack() as ctx:
        tc = ctx.enter_context(tile.TileContext(nc))
        xp = ctx.enter_context(tc.tile_pool(name="x", bufs=bufs))
        qp = ctx.enter_context(tc.tile_pool(name="q", bufs=7))
        trp = ctx.enter_context(tc.tile_pool(name="tr", bufs=4))
        mp_max = ctx.enter_context(tc.tile_pool(name="m", bufs=8))
        mrp = ctx.enter_context(tc.tile_pool(name="mrep", bufs=7))
        mp = ctx.enter_context(tc.tile_pool(name="mask", bufs=4))
        singles = ctx.enter_context(tc.tile_pool(name="singles", bufs=1))
        psum = ctx.enter_context(tc.tile_pool(name="psum", bufs=1, space="PSUM"))

        # per-sample matmul selectors: sel[:, s, :] is [128, 8] with col s = 1
        sel = singles.tile([P, SPB, SPB], bf16)
        nc.vector.memset(sel, 0.0)
        for s in range(SPB):
            nc.vector.memset(sel[:, s, s : s + 1], 1.0)

        # Warm the ScalarE Sign activation table (~2.7 us load+drain)
        # before the first real mask depends on it.
        warm = singles.tile([P, 2], fp32)
        nc.vector.memset(warm[:, 0:1], 0.0)
        nc.scalar.activation(
            out=warm[:, 1:2],
            in_=warm[:, 0:1],
            func=mybir.ActivationFunctionType.Sign,
        )

        acc = psum.tile([SPB, QCHUNK], fp32)  # one PSUM bank, [8, 400]

        total_mm = sum(np_ * C // (P * QCHUNK) for _, _, np_, _ in units)
        nu = len(units)
        xts = [None] * nu
        qts = [None] * nu
        m3s = [None] * nu
        mr4s = [None] * nu

        def issue_load(i):
            s, n0, npos, _ = units[i]
            k = npos // P
            f = k * C
            xt = xp.tile([P, f], fp32, tag="x")
            src = x_in[s, n0 : n0 + npos, :].rearrange("(p k) c -> p (k c)", p=P)
            nc.sync.dma_start(out=xt, in_=src)
            xts[i] = xt

        def issue_prep(i):
            """Construct (B) + reduce; engine streams stay mostly parallel."""
            s, n0, npos, fl = units[i]
            k = npos // P
            f = k * C
            xt = xts[i]
            if fl == "b":
                qt = qp.tile([P, f], i16, tag="q")
                nc.scalar.activation(
                    out=qt,
                    in_=xt,
                    func=mybir.ActivationFunctionType.Copy,
                    scale=QSCALE,
                )
                qts[i] = qt
                # 2x max tree: contiguous half-splits keep operands packed
                q3 = qt.rearrange("p (k c) -> p k c", c=C)
                r1 = trp.tile([P, k, 50], i16, tag="r1")
                nc.vector.tensor_tensor(
                    out=r1,
                    in0=q3[:, :, 0:50],
                    in1=q3[:, :, 50:100],
                    op=mybir.AluOpType.max,
                )
                r2 = trp.tile([P, k, 25], i16, tag="r2")
                nc.vector.tensor_tensor(
                    out=r2,
                    in0=r1[:, :, 0:25],
                    in1=r1[:, :, 25:50],
                    op=mybir.AluOpType.max,
                )
                m3 = mp_max.tile([P, k, 1], i16, tag="m")
                nc.vector.tensor_reduce(
                    out=m3,
                    in_=r2,
                    axis=mybir.AxisListType.X,
                    op=mybir.AluOpType.max,
                )
                m3s[i] = m3
                # packed 4x replica of the per-group max (tiny DVE op;
                # GpSimd was tried here and cost +25us — its software ops
                # serialize against the DVE stream)
                mr4 = mrp.tile([P, k, 4], i16, tag="mr")
                nc.vector.tensor_scalar(
                    out=mr4,
                    in0=m3.broadcast_to([P, k, 4]),
                    scalar1=0,
                    scalar2=None,
                    op0=mybir.AluOpType.add,
                )
                mr4s[i] = mr4
            else:
                m3 = mp_max.tile([P, k, 1], fp32, tag="m")
                nc.vector.tensor_reduce(
                    out=m3,
                    in_=xt.rearrange("p (k c) -> p k c", c=C),
                    axis=mybir.AxisListType.X,
                    op=mybir.AluOpType.max,
                )
                m3s[i] = m3

        LA_DMA = 8
        LA_PREP = 5
        for i in range(min(LA_DMA, nu)):
            issue_load(i)
        for i in range(min(LA_PREP, nu)):
            issue_prep(i)

        mm = 0
        for i in range(nu):
            if i + LA_DMA < nu:
                issue_load(i + LA_DMA)
            if i + LA_PREP < nu:
                issue_prep(i + LA_PREP)
            s, n0, npos, fl = units[i]
            k = npos // P
            f = k * C
            nq = f // QCHUNK
            m3 = m3s[i]

            mask = mp.tile([P, f], bf16, tag="mask")
            if fl == "b":
                qt = qts[i]
                q4 = qt.rearrange("p (k c4 j) -> p k c4 j", k=k, c4=C // 4, j=4)
                mask4 = mask.rearrange(
                    "p (k c4 j) -> p k c4 j", k=k, c4=C // 4, j=4
                )
                mr4 = mr4s[i]
                m_b = mr4.rearrange("p (k one) j -> p k one j", one=1).broadcast_to(
                    [P, k, C // 4, 4]
                )
                nc.vector.tensor_tensor(
                    out=mask4,
                    in0=q4,
                    in1=m_b,
                    op=mybir.AluOpType.is_lt,
                )
            elif fl == "a":
                xt = xts[i]
                x3 = xt.rearrange("p (k c) -> p k c", c=C)
                mask3 = mask.rearrange("p (k c) -> p k c", c=C)
                for j in range(k):
                    nc.scalar.activation(
                        out=mask3[:, j, :],
                        in_=x3[:, j, :],
                        func=mybir.ActivationFunctionType.Sign,
                        bias=m3[:, j, :],
                        scale=-1.0,
                    )
            else:  # d: fp32 DVE TT
                xt = xts[i]
                x3 = xt.rearrange("p (k c) -> p k c", c=C)
                mask3 = mask.rearrange("p (k c) -> p k c", c=C)
                m_b = m3.broadcast_to([P, k, C])
                nc.vector.tensor_tensor(
                    out=mask3,
                    in0=x3,
                    in1=m_b,
                    op=mybir.AluOpType.is_lt,
                )

            for q in range(nq):
                nc.tensor.matmul(
                    acc,
                    sel[:, s, :],
                    mask[:, q * QCHUNK : (q + 1) * QCHUNK],
                    start=(mm == 0),
                    stop=(mm == total_mm - 1),
                )
                mm += 1

        # ---- finale: fold the 4 k-subgroups, freqs = 1 - S/N ----
        S = singles.tile([SPB, C], fp32)
        nc.vector.tensor_reduce(
            out=S,
            in_=acc.rearrange("p (g c) -> p c g", c=C),
            axis=mybir.AxisListType.X,
            op=mybir.AluOpType.add,
        )

        fq = singles.tile([SPB, C], fp32)
        nc.vector.tensor_scalar(
            out=fq,
            in0=S,
            scalar1=-1.0 / N,
            scalar2=1.0,
            op0=mybir.AluOpType.mult,
            op1=mybir.AluOpType.add,
        )

        nc.sync.dma_start(out=out_d[:, :], in_=fq)

    nc.finalize()
    return nc


_NC_CACHE = None


def _get_nc():
    global _NC_CACHE
    if _NC_CACHE is None:
        _NC_CACHE = build_bass()
    return _NC_CACHE


def run(inputs: dict, trace: bool = False, nc=None):
    """Shard, run on 8 cores, gather. Returns (freqs [64,100] f32, results)."""
    x = np.ascontiguousarray(np.asarray(inputs["input"], dtype=np.float32))
    assert x.shape == (B, N, C), x.shape
    if nc is None:
        nc = _get_nc()
    in_maps = [
        {"input": x[core * SPB : (core + 1) * SPB]} for core in range(NCORES)
    ]
    res = run_bass_kernel_spmd(nc, in_maps, list(range(NCORES)), trace=trace)
    out = np.concatenate([res.results[core]["freqs"] for core in range(NCORES)], axis=0)
    return out.astype(np.float32), res


def kernel(**inputs) -> np.ndarray:
    out, _ = run(inputs)
    return out


# revision 19
# speedup vs baseline: 1.3002x; 1.1424x over previous
"""Trainium2 Bass kernel for per-sample argmax-histogram (nn_BasicCount).

Input : full  x [64, 16384, 100] f32
Output: full  freqs [64, 100] f32  (per-sample normalized histogram of
        argmax over classes)

Sharding: pure data parallel — batch dim split 8 ways across the 8
NeuronCores (8 samples per core), no communication.

v2 algorithm (per core, shapes hardcoded):
  Work units of up to 4096 positions laid out [128 partitions x k groups
  x 100 classes].  Two tile flavors:

  Flavor B (int16 2x path, most tiles):
    1. ACT: q = int16(x * 4096) as ONE full-tile Copy instruction
       (monotone quantization; quantum 2.44e-4 -> false-tie rate
       ~3e-4/position, ~0.05 expected stray counts per bin — far
       under the 2e-2 rel-err gate).
    2. DVE: m3i[p,k] = max_c q — segmented tensor_reduce on int16.
    3. DVE (tiny): mrep4[p, k*4+j] = m3i[p,k] (packed 4x replica).
    4. DVE: mask = is_lt(q, mrep4) viewed [P, k, 25, 4] so every
       operand's innermost AP dim is packed 2-byte stride-1 -> DVE
       2x mode (the m-broadcast rides a middle 0-stride dim).
  Flavor A (legacy ACT path, for engine balance):
    1. DVE: m3 = tensor_reduce(max) on fp32 x.
    2. ACT: per-group Sign(m - x) instrs (FD=100, ~271 ns each).
  Both produce a bf16 complement mask [x < m]; PE accumulates
  per-sample mask sums into one PSUM bank [8, 400]; finale folds the
  4 k-subgroups, freqs = 1 - S/N.

Engine model (HW-measured baseline): DVE 1x = (FD+~130)/0.96 ns; ACT
per-instr = (FD+~160)/0.96 ns; DMA union busy 152.8 us (the roofline).
Flavor B: ACT 3.4us + DVE ~5.5us per tile; flavor A: ACT 8.67, DVE
3.45.  Mix ~7 A-tiles to balance both engines near the DMA floor.

Dead ends (HW-measured, do not revisit): GpSimd offload (SBUF
contention halves DVE), fp32 PE subtract (PE critical), bf16/fp16
value compares (false argmax ties above the 2e-2 gate), custom DVE ops
(1x only).  int16 compares are exact on the quantized grid, so the
2e-2 risk analysis is Poisson tie-counting, not fp rounding.
"""

import sys

if "/opt/trn_rl_repo" not in sys.path:
    sys.path.insert(0, "/opt/trn_rl_repo")

from contextlib import ExitStack

import numpy as np

import concourse.bacc as bacc
import concourse.bass as bass
import concourse.tile as tile
from concourse import mybir
from concourse.bass_utils import run_bass_kernel_spmd

B, N, C = 64, 16384, 100
NCORES = 8
SPB = B // NCORES  # samples per core = 8
P = 128  # partitions
POS_PER_TILE = 4096
K = POS_PER_TILE // P  # position groups per partition = 32
F = K * C  # free size per tile = 3200
TILES_PER_SAMPLE = N // POS_PER_TILE  # 4
NTILES = SPB * TILES_PER_SAMPLE  # 32
QCHUNK = 400  # matmul rhs free chunk (4 groups x 100 classes)
QSCALE = 4096.0  # int16 quantization scale; |x| < 8 guaranteed


def _schedule(variant):
    """Work units: (sample, n0, npos, flavor); flavor in {a, b, d}.

    a = ACT per-group Sign masks (fp32 exact)
    b = int16 2x DVE masks (ACT constructs q)
    d = fp32 DVE TT is_lt masks (legacy dve flavor)
    """
    units = []
    SUB = 1024
    if variant == "legacy":
        mid_pat = [
            "a", "d", "a", "d", "a", "a", "d", "a", "d", "a",
            "a", "d", "a", "d", "a", "a", "d", "a", "d", "a",
            "a", "d", "a", "d", "a", "a", "d", "d", "a", "d",
        ]
        ramp = ["a", "d", "a", "d"]
        drain = ["d", "d", "d", "d"]
    elif variant == "allb":
        mid_pat = ["b"] * 30
        ramp = ["b"] * 4
        drain = ["b"] * 4
    elif variant == "mix7":  # 7 A-tiles spread among B-tiles
        mid_pat = list("bbabbabbbabbbabbabbbabbabbbbbb")
        ramp = ["b", "a", "b", "b"]
        drain = ["b"] * 4
    else:  # v2 default: uniform all-B pipeline.  A-tile mixing was
        # HW-measured SLOWER (183.6 vs 178.0): each A-tile's 8.8us ACT
        # Sign block sits between DMA arrival and the next construct,
        # blowing ~2us holes in the DVE stream that outweigh the ~0.9us
        # balance gain.
        mid_pat = ["b"] * 30
        ramp = ["b"] * 4
        drain = ["b"] * 4
    assert len(mid_pat) == 30
    for j, e in enumerate(ramp):
        units.append((0, j * SUB, SUB, e))
    for i in range(1, NTILES - 1):
        s = i // TILES_PER_SAMPLE
        n0 = (i % TILES_PER_SAMPLE) * POS_PER_TILE
        units.append((s, n0, POS_PER_TILE, mid_pat[i - 1]))
    base = (TILES_PER_SAMPLE - 1) * POS_PER_TILE
    for j, e in enumerate(drain):
        units.append((SPB - 1, base + j * SUB, SUB, e))
    return units


def build_bass(variant: str = "v2", bufs: int = 9):
    fp32 = mybir.dt.float32
    bf16 = mybir.dt.bfloat16
    i16 = mybir.dt.int16

    units = _schedule(variant)

    nc = bacc.Bacc(None)
    x_in = nc.declare_dram_parameter("input", [SPB, N, C], fp32, isOutput=False)
    out_d = nc.declare_dram_parameter("freqs", [SPB, C], fp32, isOutput=True)

    with ExitStack() as ctx:
        tc = ctx.enter_context(tile.TileContext(nc))
        xp = ctx.enter_context(tc.tile_pool(name="x", bufs=bufs))
        qp = ctx.enter_context(tc.tile_pool(name="q", bufs=7))
        trp = ctx.enter_context(tc.tile_pool(name="tr", bufs=4))
        mp_max = ctx.enter_context(tc.tile_pool(name="m", bufs=8))
        mrp = ctx.enter_context(tc.tile_pool(name="mrep", bufs=7))
        mp = ctx.enter_context(tc.tile_pool(name="mask", bufs=4))
        singles = ctx.enter_context(tc.tile_pool(name="singles", bufs=1))
        psum = ctx.enter_context(tc.tile_pool(name="psum", bufs=1, space="PSUM"))

        # per-sample matmul selectors: sel[:, s, :] is [128, 8] with col s = 1
        sel = singles.tile([P, SPB, SPB], bf16)
        nc.vector.memset(sel, 0.0)
        for s in range(SPB):
            nc.vector.memset(sel[:, s, s : s + 1], 1.0)

        # Warm the ScalarE Sign activation table (~2.7 us load+drain)
        # before the first real mask depends on it.
        warm = singles.tile([P, 2], fp32)
        nc.vector.memset(warm[:, 0:1], 0.0)
        nc.scalar.activation(
            out=warm[:, 1:2],
            in_=warm[:, 0:1],
            func=mybir.ActivationFunctionType.Sign,
        )

        acc = psum.tile([SPB, QCHUNK], fp32)  # one PSUM bank, [8, 400]

        total_mm = sum(np_ * C // (P * QCHUNK) for _, _, np_, _ in units)
        nu = len(units)
        xts = [None] * nu
        qts = [None] * nu
        m3s = [None] * nu
        mr4s = [None] * nu

        def issue_load(i):
            s, n0, npos, _ = units[i]
            k = npos // P
            f = k * C
            xt = xp.tile([P, f], fp32, tag="x")
            src = x_in[s, n0 : n0 + npos, :].rearrange("(p k) c -> p (k c)", p=P)
            nc.sync.dma_start(out=xt, in_=src)
            xts[i] = xt

        def issue_prep(i):
            """Construct (B) + reduce; engine streams stay mostly parallel."""
            s, n0, npos, fl = units[i]
            k = npos // P
            f = k * C
            xt = xts[i]
            if fl == "b":
                qt = qp.tile([P, f], i16, tag="q")
                nc.scalar.activation(
                    out=qt,
                    in_=xt,
                    func=mybir.ActivationFunctionType.Copy,
                    scale=QSCALE,
                )
                qts[i] = qt
                # 2x max tree: contiguous half-splits keep operands packed
                q3 = qt.rearrange("p (k c) -> p k c", c=C)
                r1 = trp.tile([P, k, 50], i16, tag="r1")
                nc.vector.tensor_tensor(
                    out=r1,
                    in0=q3[:, :, 0:50],
                    in1=q3[:, :, 50:100],
                    op=mybir.AluOpType.max,
                )
                r2 = trp.tile([P, k, 25], i16, tag="r2")
                nc.vector.tensor_tensor(
                    out=r2,
                    in0=r1[:, :, 0:25],
                    in1=r1[:, :, 25:50],
                    op=mybir.AluOpType.max,
                )
                m3 = mp_max.tile([P, k, 1], i16, tag="m")
                nc.vector.tensor_reduce(
                    out=m3,
                    in_=r2,
                    axis=mybir.AxisListType.X,
                    op=mybir.AluOpType.max,
                )
                m3s[i] = m3
            else:
                m3 = mp_max.tile([P, k, 1], fp32, tag="m")
                nc.vector.tensor_reduce(
                    out=m3,
                    in_=xt.rearrange("p (k c) -> p k c", c=C),
                    axis=mybir.AxisListType.X,
                    op=mybir.AluOpType.max,
                )
                m3s[i] = m3

        def issue_mrep(i):
            """4x packed replica of the per-group max, on ACT (it has
            slack; keeps the op off the bottleneck DVE stream).  Issued
            ahead of any prep construct so a DMA-blocked construct can't
            sit in front of it in ACT's in-order queue."""
            if units[i][3] != "b":
                return
            k = units[i][2] // P
            mr4 = mrp.tile([P, k, 4], i16, tag="mr")
            nc.scalar.activation(
                out=mr4,
                in_=m3s[i].broadcast_to([P, k, 4]),
                func=mybir.ActivationFunctionType.Copy,
                scale=1.0,
            )
            mr4s[i] = mr4

        LA_DMA = 8
        LA_PREP = 5
        LA_MREP = 2
        for i in range(min(LA_DMA, nu)):
            issue_load(i)
        for i in range(min(LA_PREP, nu)):
            issue_prep(i)
        for i in range(min(LA_MREP, nu)):
            issue_mrep(i)

        mm = 0
        for i in range(nu):
            # mask-phase ops first: they are ready now, so a prep op
            # blocked on a not-yet-arrived DMA tile can't starve them
            # in the in-order engine queues
            s, n0, npos, fl = units[i]
            k = npos // P
            f = k * C
            nq = f // QCHUNK
            m3 = m3s[i]

            mask = mp.tile([P, f], bf16, tag="mask")
            if fl == "b":
                qt = qts[i]
                q4 = qt.rearrange("p (k c4 j) -> p k c4 j", k=k, c4=C // 4, j=4)
                mask4 = mask.rearrange(
                    "p (k c4 j) -> p k c4 j", k=k, c4=C // 4, j=4
                )
                mr4 = mr4s[i]
                m_b = mr4.rearrange("p (k one) j -> p k one j", one=1).broadcast_to(
                    [P, k, C // 4, 4]
                )
                nc.vector.tensor_tensor(
                    out=mask4,
                    in0=q4,
                    in1=m_b,
                    op=mybir.AluOpType.is_lt,
                )
            elif fl == "a":
                xt = xts[i]
                x3 = xt.rearrange("p (k c) -> p k c", c=C)
                mask3 = mask.rearrange("p (k c) -> p k c", c=C)
                for j in range(k):
                    nc.scalar.activation(
                        out=mask3[:, j, :],
                        in_=x3[:, j, :],
                        func=mybir.ActivationFunctionType.Sign,
                        bias=m3[:, j, :],
                        scale=-1.0,
                    )
            else:  # d: fp32 DVE TT
                xt = xts[i]
                x3 = xt.rearrange("p (k c) -> p k c", c=C)
                mask3 = mask.rearrange("p (k c) -> p k c", c=C)
                m_b = m3.broadcast_to([P, k, C])
                nc.vector.tensor_tensor(
                    out=mask3,
                    in0=x3,
                    in1=m_b,
                    op=mybir.AluOpType.is_lt,
                )

            for q in range(nq):
                nc.tensor.matmul(
                    acc,
                    sel[:, s, :],
                    mask[:, q * QCHUNK : (q + 1) * QCHUNK],
                    start=(mm == 0),
                    stop=(mm == total_mm - 1),
                )
                mm += 1

            if i + LA_MREP < nu:
                issue_mrep(i + LA_MREP)
            if i + LA_PREP < nu:
                issue_prep(i + LA_PREP)
            if i + LA_DMA < nu:
                issue_load(i + LA_DMA)

        # ---- finale: fold the 4 k-subgroups, freqs = 1 - S/N ----
        S = singles.tile([SPB, C], fp32)
        nc.vector.tensor_reduce(
            out=S,
            in_=acc.rearrange("p (g c) -> p c g", c=C),
            axis=mybir.AxisListType.X,
            op=mybir.AluOpType.add,
        )

        fq = singles.tile([SPB, C], fp32)
        nc.vector.tensor_scalar(
            out=fq,
            in0=S,
            scalar1=-1.0 / N,
            scalar2=1.0,
            op0=mybir.AluOpType.mult,
            op1=mybir.AluOpType.add,
        )

        nc.sync.dma_start(out=out_d[:, :], in_=fq)

    nc.finalize()
    return nc


_NC_CACHE = None


def _get_nc():
    global _NC_CACHE
    if _NC_CACHE is None:
        _NC_CACHE = build_bass()
    return _NC_CACHE


def run(inputs: dict, trace: bool = False, nc=None):
    """Shard, run on 8 cores, gather. Returns (freqs [64,100] f32, results)."""
    x = np.ascontiguousarray(np.asarray(inputs["input"], dtype=np.float32))
    assert x.shape == (B, N, C), x.shape
    if nc is None:
        nc = _get_nc()
    in_maps = [
        {"input": x[core * SPB : (core + 1) * SPB]} for core in range(NCORES)
    ]
    res = run_bass_kernel_spmd(nc, in_maps, list(range(NCORES)), trace=trace)
    out = np.concatenate([res.results[core]["freqs"] for core in range(NCORES)], axis=0)
    return out.astype(np.float32), res


def kernel(**inputs) -> np.ndarray:
    out, _ = run(inputs)
    return out


# revision 22
# speedup vs baseline: 1.3020x; 1.0014x over previous
"""Trainium2 Bass kernel for per-sample argmax-histogram (nn_BasicCount).

Input : full  x [64, 16384, 100] f32
Output: full  freqs [64, 100] f32  (per-sample normalized histogram of
        argmax over classes)

Sharding: pure data parallel — batch dim split 8 ways across the 8
NeuronCores (8 samples per core), no communication.

v2 algorithm (per core, shapes hardcoded):
  Work units of up to 4096 positions laid out [128 partitions x k groups
  x 100 classes].  Two tile flavors:

  Flavor B (int16 2x path, most tiles):
    1. ACT: q = int16(x * 4096) as ONE full-tile Copy instruction
       (monotone quantization; quantum 2.44e-4 -> false-tie rate
       ~3e-4/position, ~0.05 expected stray counts per bin — far
       under the 2e-2 rel-err gate).
    2. DVE: m3i[p,k] = max_c q — segmented tensor_reduce on int16.
    3. DVE (tiny): mrep4[p, k*4+j] = m3i[p,k] (packed 4x replica).
    4. DVE: mask = is_lt(q, mrep4) viewed [P, k, 25, 4] so every
       operand's innermost AP dim is packed 2-byte stride-1 -> DVE
       2x mode (the m-broadcast rides a middle 0-stride dim).
  Flavor A (legacy ACT path, for engine balance):
    1. DVE: m3 = tensor_reduce(max) on fp32 x.
    2. ACT: per-group Sign(m - x) instrs (FD=100, ~271 ns each).
  Both produce a bf16 complement mask [x < m]; PE accumulates
  per-sample mask sums into one PSUM bank [8, 400]; finale folds the
  4 k-subgroups, freqs = 1 - S/N.

Engine model (HW-measured baseline): DVE 1x = (FD+~130)/0.96 ns; ACT
per-instr = (FD+~160)/0.96 ns; DMA union busy 152.8 us (the roofline).
Flavor B: ACT 3.4us + DVE ~5.5us per tile; flavor A: ACT 8.67, DVE
3.45.  Mix ~7 A-tiles to balance both engines near the DMA floor.

Dead ends (HW-measured, do not revisit): GpSimd offload (SBUF
contention halves DVE), fp32 PE subtract (PE critical), bf16/fp16
value compares (false argmax ties above the 2e-2 gate), custom DVE ops
(1x only).  int16 compares are exact on the quantized grid, so the
2e-2 risk analysis is Poisson tie-counting, not fp rounding.
"""

import sys

if "/opt/trn_rl_repo" not in sys.path:
    sys.path.insert(0, "/opt/trn_rl_repo")

from contextlib import ExitStack

import numpy as np

import concourse.bacc as bacc
import concourse.bass as bass
import concourse.tile as tile
from concourse import mybir
from concourse.bass_utils import run_bass_kernel_spmd

B, N, C = 64, 16384, 100
NCORES = 8
SPB = B // NCORES  # samples per core = 8
P = 128  # partitions
POS_PER_TILE = 4096
K = POS_PER_TILE // P  # position groups per partition = 32
F = K * C  # free size per tile = 3200
TILES_PER_SAMPLE = N // POS_PER_TILE  # 4
NTILES = SPB * TILES_PER_SAMPLE  # 32
QCHUNK = 400  # matmul rhs free chunk (4 groups x 100 classes)
QSCALE = 4096.0  # int16 quantization scale; |x| < 8 guaranteed


def _schedule(variant):
    """Work units: (sample, n0, npos, flavor); flavor in {a, b, d}.

    a = ACT per-group Sign masks (fp32 exact)
    b = int16 2x DVE masks (ACT constructs q)
    d = fp32 DVE TT is_lt masks (legacy dve flavor)
    """
    units = []
    SUB = 1024
    if variant == "legacy":
        mid_pat = [
            "a", "d", "a", "d", "a", "a", "d", "a", "d", "a",
            "a", "d", "a", "d", "a", "a", "d", "a", "d", "a",
            "a", "d", "a", "d", "a", "a", "d", "d", "a", "d",
        ]
        ramp = ["a", "d", "a", "d"]
        drain = ["d", "d", "d", "d"]
    elif variant == "allb":
        mid_pat = ["b"] * 30
        ramp = ["b"] * 4
        drain = ["b"] * 4
    elif variant == "mix7":  # 7 A-tiles spread among B-tiles
        mid_pat = list("bbabbabbbabbbabbabbbabbabbbbbb")
        ramp = ["b", "a", "b", "b"]
        drain = ["b"] * 4
    else:  # v2 default: uniform all-B pipeline.  A-tile mixing was
        # HW-measured SLOWER (183.6 vs 178.0): each A-tile's 8.8us ACT
        # Sign block sits between DMA arrival and the next construct,
        # blowing ~2us holes in the DVE stream that outweigh the ~0.9us
        # balance gain.  First TWO tiles split into subtiles: finer
        # grain while the DMA runahead lead builds.
        for t in range(2):
            for j in range(4):
                units.append((0, t * POS_PER_TILE + j * SUB, SUB, "b"))
        for i in range(2, NTILES - 1):
            s = i // TILES_PER_SAMPLE
            n0 = (i % TILES_PER_SAMPLE) * POS_PER_TILE
            units.append((s, n0, POS_PER_TILE, "b"))
        base = (TILES_PER_SAMPLE - 1) * POS_PER_TILE
        for j in range(4):
            units.append((SPB - 1, base + j * SUB, SUB, "b"))
        return units
    assert len(mid_pat) == 30
    for j, e in enumerate(ramp):
        units.append((0, j * SUB, SUB, e))
    for i in range(1, NTILES - 1):
        s = i // TILES_PER_SAMPLE
        n0 = (i % TILES_PER_SAMPLE) * POS_PER_TILE
        units.append((s, n0, POS_PER_TILE, mid_pat[i - 1]))
    base = (TILES_PER_SAMPLE - 1) * POS_PER_TILE
    for j, e in enumerate(drain):
        units.append((SPB - 1, base + j * SUB, SUB, e))
    return units


def build_bass(variant: str = "v2", bufs: int = 7):
    fp32 = mybir.dt.float32
    bf16 = mybir.dt.bfloat16
    i16 = mybir.dt.int16

    units = _schedule(variant)

    nc = bacc.Bacc(None)
    x_in = nc.declare_dram_parameter("input", [SPB, N, C], fp32, isOutput=False)
    out_d = nc.declare_dram_parameter("freqs", [SPB, C], fp32, isOutput=True)

    with ExitStack() as ctx:
        tc = ctx.enter_context(tile.TileContext(nc))
        xp = ctx.enter_context(tc.tile_pool(name="x", bufs=bufs))
        qp = ctx.enter_context(tc.tile_pool(name="q", bufs=10))
        trp = ctx.enter_context(tc.tile_pool(name="tr", bufs=4))
        mp_max = ctx.enter_context(tc.tile_pool(name="m", bufs=8))
        mrp = ctx.enter_context(tc.tile_pool(name="mrep", bufs=7))
        mp = ctx.enter_context(tc.tile_pool(name="mask", bufs=4))
        singles = ctx.enter_context(tc.tile_pool(name="singles", bufs=1))
        psum = ctx.enter_context(tc.tile_pool(name="psum", bufs=1, space="PSUM"))

        # per-sample matmul selectors: sel[:, s, :] is [128, 8] with col s = 1
        sel = singles.tile([P, SPB, SPB], bf16)
        nc.vector.memset(sel, 0.0)
        for s in range(SPB):
            nc.vector.memset(sel[:, s, s : s + 1], 1.0)

        # Warm the ScalarE Sign activation table (~2.7 us load+drain)
        # before the first real mask depends on it.
        warm = singles.tile([P, 2], fp32)
        nc.vector.memset(warm[:, 0:1], 0.0)
        nc.scalar.activation(
            out=warm[:, 1:2],
            in_=warm[:, 0:1],
            func=mybir.ActivationFunctionType.Sign,
        )

        acc = psum.tile([SPB, QCHUNK], fp32)  # one PSUM bank, [8, 400]

        total_mm = sum(np_ * C // (P * QCHUNK) for _, _, np_, _ in units)
        nu = len(units)
        xts = [None] * nu
        qts = [None] * nu
        m3s = [None] * nu
        mr4s = [None] * nu

        def issue_load(i):
            s, n0, npos, _ = units[i]
            k = npos // P
            f = k * C
            xt = xp.tile([P, f], fp32, tag="x")
            src = x_in[s, n0 : n0 + npos, :].rearrange("(p k) c -> p (k c)", p=P)
            nc.sync.dma_start(out=xt, in_=src)
            xts[i] = xt

        def issue_prep(i):
            """Construct (B) + reduce; engine streams stay mostly parallel."""
            s, n0, npos, fl = units[i]
            k = npos // P
            f = k * C
            xt = xts[i]
            if fl == "b":
                qt = qp.tile([P, f], i16, tag="q")
                nc.scalar.activation(
                    out=qt,
                    in_=xt,
                    func=mybir.ActivationFunctionType.Copy,
                    scale=QSCALE,
                )
                qts[i] = qt
                # 2x max tree: contiguous half-splits keep operands packed
                q3 = qt.rearrange("p (k c) -> p k c", c=C)
                r1 = trp.tile([P, k, 50], i16, tag="r1")
                nc.vector.tensor_tensor(
                    out=r1,
                    in0=q3[:, :, 0:50],
                    in1=q3[:, :, 50:100],
                    op=mybir.AluOpType.max,
                )
                r2 = trp.tile([P, k, 25], i16, tag="r2")
                nc.vector.tensor_tensor(
                    out=r2,
                    in0=r1[:, :, 0:25],
                    in1=r1[:, :, 25:50],
                    op=mybir.AluOpType.max,
                )
                m3 = mp_max.tile([P, k, 1], i16, tag="m")
                nc.vector.tensor_reduce(
                    out=m3,
                    in_=r2,
                    axis=mybir.AxisListType.X,
                    op=mybir.AluOpType.max,
                )
                m3s[i] = m3
            else:
                m3 = mp_max.tile([P, k, 1], fp32, tag="m")
                nc.vector.tensor_reduce(
                    out=m3,
                    in_=xt.rearrange("p (k c) -> p k c", c=C),
                    axis=mybir.AxisListType.X,
                    op=mybir.AluOpType.max,
                )
                m3s[i] = m3

        def issue_mrep(i):
            """4x packed replica of the per-group max, on ACT (it has
            slack; keeps the op off the bottleneck DVE stream).  Issued
            ahead of any prep construct so a DMA-blocked construct can't
            sit in front of it in ACT's in-order queue."""
            if units[i][3] != "b":
                return
            k = units[i][2] // P
            mr4 = mrp.tile([P, k, 4], i16, tag="mr")
            nc.scalar.activation(
                out=mr4,
                in_=m3s[i].broadcast_to([P, k, 4]),
                func=mybir.ActivationFunctionType.Copy,
                scale=1.0,
            )
            mr4s[i] = mr4

        LA_DMA = 8
        LA_PREP = 5
        LA_MREP = 2
        for i in range(min(LA_DMA, nu)):
            issue_load(i)
        for i in range(min(LA_PREP, nu)):
            issue_prep(i)
        for i in range(min(LA_MREP, nu)):
            issue_mrep(i)

        mm = 0
        for i in range(nu):
            # mask-phase ops first: they are ready now, so a prep op
            # blocked on a not-yet-arrived DMA tile can't starve them
            # in the in-order engine queues
            s, n0, npos, fl = units[i]
            k = npos // P
            f = k * C
            nq = f // QCHUNK
            m3 = m3s[i]

            mask = mp.tile([P, f], bf16, tag="mask")
            if fl == "b":
                qt = qts[i]
                q4 = qt.rearrange("p (k c4 j) -> p k c4 j", k=k, c4=C // 4, j=4)
                mask4 = mask.rearrange(
                    "p (k c4 j) -> p k c4 j", k=k, c4=C // 4, j=4
                )
                mr4 = mr4s[i]
                m_b = mr4.rearrange("p (k one) j -> p k one j", one=1).broadcast_to(
                    [P, k, C // 4, 4]
                )
                nc.vector.tensor_tensor(
                    out=mask4,
                    in0=q4,
                    in1=m_b,
                    op=mybir.AluOpType.is_lt,
                )
            elif fl == "a":
                xt = xts[i]
                x3 = xt.rearrange("p (k c) -> p k c", c=C)
                mask3 = mask.rearrange("p (k c) -> p k c", c=C)
                for j in range(k):
                    nc.scalar.activation(
                        out=mask3[:, j, :],
                        in_=x3[:, j, :],
                        func=mybir.ActivationFunctionType.Sign,
                        bias=m3[:, j, :],
                        scale=-1.0,
                    )
            else:  # d: fp32 DVE TT
                xt = xts[i]
                x3 = xt.rearrange("p (k c) -> p k c", c=C)
                mask3 = mask.rearrange("p (k c) -> p k c", c=C)
                m_b = m3.broadcast_to([P, k, C])
                nc.vector.tensor_tensor(
                    out=mask3,
                    in0=x3,
                    in1=m_b,
                    op=mybir.AluOpType.is_lt,
                )

            for q in range(nq):
                nc.tensor.matmul(
                    acc,
                    sel[:, s, :],
                    mask[:, q * QCHUNK : (q + 1) * QCHUNK],
                    start=(mm == 0),
                    stop=(mm == total_mm - 1),
                )
                mm += 1

            if i + LA_MREP < nu:
                issue_mrep(i + LA_MREP)
            if i + LA_PREP < nu:
                issue_prep(i + LA_PREP)
            if i + LA_DMA < nu:
                issue_load(i + LA_DMA)

        # ---- finale: fold the 4 k-subgroups, freqs = 1 - S/N ----
        S = singles.tile([SPB, C], fp32)
        nc.vector.tensor_reduce(
            out=S,
            in_=acc.rearrange("p (g c) -> p c g", c=C),
            axis=mybir.AxisListType.X,
            op=mybir.AluOpType.add,
        )

        fq = singles.tile([SPB, C], fp32)
        nc.vector.tensor_scalar(
            out=fq,
            in0=S,
            scalar1=-1.0 / N,
            scalar2=1.0,
            op0=mybir.AluOpType.mult,
            op1=mybir.AluOpType.add,
        )

        nc.sync.dma_start(out=out_d[:, :], in_=fq)

    nc.finalize()
    return nc


_NC_CACHE = None


def _get_nc():
    global _NC_CACHE
    if _NC_CACHE is None:
        _NC_CACHE = build_bass()
    return _NC_CACHE


def run(inputs: dict, trace: bool = False, nc=None):
    """Shard, run on 8 cores, gather. Returns (freqs [64,100] f32, results)."""
    x = np.ascontiguousarray(np.asarray(inputs["input"], dtype=np.float32))
    assert x.shape == (B, N, C), x.shape
    if nc is None:
        nc = _get_nc()
    in_maps = [
        {"input": x[core * SPB : (core + 1) * SPB]} for core in range(NCORES)
    ]
    res = run_bass_kernel_spmd(nc, in_maps, list(range(NCORES)), trace=trace)
    out = np.concatenate([res.results[core]["freqs"] for core in range(NCORES)], axis=0)
    return out.astype(np.float32), res


def kernel(**inputs) -> np.ndarray:
    out, _ = run(inputs)
    return out
